# revision 1
# baseline (speedup 1.0000x reference)
"""DeltaNet forward kernel for Trainium2, sharded over 8 NeuronCores.

Sharding: core c handles batch c//2 and head-pair c%2 (heads {2*(c%2), 2*(c%2)+1}).
Each core computes: causal depthwise conv+silu, q/k/v/beta/g projections for its
head pair, the delta-rule recurrence via the chunked WY representation
(chunk=128, (I+A)^-1 via Neumann-series doubling), gated RMSNorm, and a partial
output projection against its 512-column slice of Wo. The host sums the two
half-DV partials per batch (row-parallel unshard).
"""

import sys

for _p in ("/opt/trn_rl_repo", "/root/.axon_site"):
    if _p not in sys.path:
        sys.path.insert(0, _p)

import numpy as np

import concourse.bass as bass
import concourse.tile as tile
from concourse import bacc, mybir
from concourse.bass_utils import run_bass_kernel_spmd
from concourse.masks import make_identity

F32 = mybir.dt.float32
F32R = mybir.dt.float32r
BF16 = mybir.dt.bfloat16

B, L, D, H = 4, 2048, 1024, 4
DK, DV = 512, 1024
HK, HV = 128, 256
CONV, EPS = 4, 1e-5
C = 128            # delta-rule chunk length
NCH = L // C       # 16 chunks
LB = 256           # L-block for projections
NLB = L // LB      # 4
KD = D // 128      # 8 contraction slices
HPC = 2            # heads per core
N_CORES = 8
QSCALE = HK ** -0.5
NEUMANN16 = True   # Tinv = sum_{k<16} M^k (else k<8)


def _mm(nc, out, lhsT, rhs, start, stop):
    """float32r matmul (full-rate 1 cycle/row when moving free dim >= 256).
    Operand tiles must be declared float32r so their producers round."""
    assert lhsT.dtype == F32R and rhs.dtype == F32R, (lhsT.dtype, rhs.dtype)
    nc.tensor.matmul(out, lhsT, rhs, start=start, stop=stop)


def build_program():
    nc = bacc.Bacc(
        "TRN2", target_bir_lowering=False, debug=False,
        enable_asserts=False, num_devices=N_CORES,
    )

    hs = nc.dram_tensor("hs", [L, D], F32, kind="ExternalInput").ap()
    cw = nc.dram_tensor("cw", [D, CONV], F32, kind="ExternalInput").ap()
    wq = nc.dram_tensor("wq", [HPC * HK, D], F32, kind="ExternalInput").ap()
    wk = nc.dram_tensor("wk", [HPC * HK, D], F32, kind="ExternalInput").ap()
    wv = nc.dram_tensor("wv", [HPC * HV, D], F32, kind="ExternalInput").ap()
    wb = nc.dram_tensor("wb", [HPC, D], F32, kind="ExternalInput").ap()
    wg = nc.dram_tensor("wg", [HPC * HV, D], F32, kind="ExternalInput").ap()
    wo = nc.dram_tensor("wo", [D, HPC * HV], F32, kind="ExternalInput").ap()
    rmsw = nc.dram_tensor("rmsw", [HV], F32, kind="ExternalInput").ap()
    y = nc.dram_tensor("y", [L, D], F32, kind="ExternalOutput").ap()

    with tile.TileContext(nc) as tc:
        _build_body(nc, tc, hs, cw, wq, wk, wv, wb, wg, wo, rmsw, y)
    nc.compile()
    return nc


def _build_body(nc, tc, hs, cw, wq, wk, wv, wb, wg, wo, rmsw, y):
    from contextlib import ExitStack

    ctx = ExitStack()
    const = ctx.enter_context(tc.tile_pool(name="const", bufs=1))
    wT = ctx.enter_context(tc.tile_pool(name="wT", bufs=1))
    wrow = ctx.enter_context(tc.tile_pool(name="wrow", bufs=3))
    dpool = ctx.enter_context(tc.tile_pool(name="dpool", bufs=2))
    ps = ctx.enter_context(tc.tile_pool(name="ps", bufs=8, space="PSUM"))
    hpool = ctx.enter_context(tc.tile_pool(name="hpool", bufs=2))
    scr = ctx.enter_context(tc.tile_pool(name="scr", bufs=3))
    xpool = ctx.enter_context(tc.tile_pool(name="xpool", bufs=2))
    hrow = ctx.enter_context(tc.tile_pool(name="hrow", bufs=2))
    qk = ctx.enter_context(tc.tile_pool(name="qk", bufs=2))
    ck = ctx.enter_context(tc.tile_pool(name="ck", bufs=3))
    ckx = ctx.enter_context(tc.tile_pool(name="ckx", bufs=6))
    otp = ctx.enter_context(tc.tile_pool(name="otp", bufs=3))
    cv = ctx.enter_context(tc.tile_pool(name="cv", bufs=3))
    sS = ctx.enter_context(tc.tile_pool(name="sS", bufs=4))
    sm = ctx.enter_context(tc.tile_pool(name="sm", bufs=6))

    # copy PSUM->SBUF on alternating engines to balance ACT/DVE load
    cp_state = [0]

    def copy_ps(dst, src):
        cp_state[0] ^= 1
        if cp_state[0]:
            nc.scalar.copy(dst, src)
        else:
            nc.vector.tensor_copy(dst, src)

    ident = const.tile([128, 128], F32)
    make_identity(nc, ident)
    epst = const.tile([128, 1], F32)
    nc.vector.memset(epst, EPS)
    identb = const.tile([128, 128], BF16)
    make_identity(nc, identb)
    # umask: 1 where free >= part (upper incl diag); numask: -1 where free > part
    umask = const.tile([128, 128], F32)
    nc.gpsimd.memset(umask, 1.0)
    nc.gpsimd.affine_select(
        out=umask, in_=umask, compare_op=mybir.AluOpType.is_ge, fill=0.0,
        base=0, channel_multiplier=-1, pattern=[[1, 128]],
    )
    numask = const.tile([128, 128], F32)
    nc.gpsimd.memset(numask, -1.0)
    nc.gpsimd.affine_select(
        out=numask, in_=numask, compare_op=mybir.AluOpType.is_gt, fill=0.0,
        base=0, channel_multiplier=-1, pattern=[[1, 128]],
    )

    def transpose_f32(in_):
        pt = ps.tile([128, 128], F32, tag="ps")
        nc.tensor.transpose(pt, in_, ident[: in_.shape[0], : in_.shape[0]])
        return pt

    # ---- constant loads ----
    cwt = const.tile([128, KD * CONV], F32)
    for d in range(KD):
        nc.sync.dma_start(
            out=cwt[:, d * CONV:(d + 1) * CONV], in_=cw[d * 128:(d + 1) * 128, :]
        )
    rmsc = const.tile([128, 2], F32)
    for s in range(2):
        nc.sync.dma_start(
            out=rmsc[:, s:s + 1],
            in_=rmsw[s * 128:(s + 1) * 128].rearrange("(p one) -> p one", one=1),
        )

    # ---- transposed weights ----
    wqT = wT.tile([128, KD, HPC * HK], F32R)   # q weights^T, pre-scaled by HK^-0.5
    wkT = wT.tile([128, KD, HPC * HK], F32R)
    wvbT = wT.tile([128, KD, HV + HPC], F32R)  # [0:256]=v head0, [256:258]=beta both
    wvT1 = wT.tile([128, KD, HV], F32R)        # v head1
    wgT = wT.tile([128, KD, HPC * HV], F32R)
    woT = wT.tile([128, 4, D], F32R)           # rms_weight folded in

    for rt in range(HPC * HK // 128):  # wq, wk: 2 row tiles each
        wr = wrow.tile([128, D], F32, tag="wrow")
        nc.sync.dma_start(out=wr, in_=wq[rt * 128:(rt + 1) * 128, :])
        wr2 = wrow.tile([128, D], F32, tag="wrow")
        nc.sync.dma_start(out=wr2, in_=wk[rt * 128:(rt + 1) * 128, :])
        for d in range(KD):
            pt = transpose_f32(wr[:, d * 128:(d + 1) * 128])
            nc.scalar.mul(wqT[:, d, rt * 128:(rt + 1) * 128], pt, QSCALE)
            pt2 = transpose_f32(wr2[:, d * 128:(d + 1) * 128])
            copy_ps(wkT[:, d, rt * 128:(rt + 1) * 128], pt2)

    for rt in range(HPC * HV // 128):  # wv: 4 row tiles
        wr = wrow.tile([128, D], F32, tag="wrow")
        nc.sync.dma_start(out=wr, in_=wv[rt * 128:(rt + 1) * 128, :])
        for d in range(KD):
            pt = transpose_f32(wr[:, d * 128:(d + 1) * 128])
            if rt < 2:
                copy_ps(wvbT[:, d, rt * 128:(rt + 1) * 128], pt)
            else:
                copy_ps(wvT1[:, d, (rt - 2) * 128:(rt - 1) * 128], pt)

    wrb = const.tile([HPC, D], F32)
    nc.sync.dma_start(out=wrb, in_=wb)
    for d in range(KD):
        pt = ps.tile([128, HPC], F32, tag="ps")
        nc.tensor.transpose(pt, wrb[:, d * 128:(d + 1) * 128], ident[:HPC, :HPC])
        copy_ps(wvbT[:, d, HV:HV + HPC], pt)

    for rt in range(HPC * HV // 128):  # wg: 4 row tiles
        wr = wrow.tile([128, D], F32, tag="wrow")
        nc.sync.dma_start(out=wr, in_=wg[rt * 128:(rt + 1) * 128, :])
        for d in range(KD):
            pt = transpose_f32(wr[:, d * 128:(d + 1) * 128])
            copy_ps(wgT[:, d, rt * 128:(rt + 1) * 128], pt)

    for rt in range(KD):  # wo: 8 row tiles of [128, 512]
        wr = wrow.tile([128, HPC * HV], F32, tag="wrow")
        nc.sync.dma_start(out=wr, in_=wo[rt * 128:(rt + 1) * 128, :])
        for s in range(4):
            pt = transpose_f32(wr[:, s * 128:(s + 1) * 128])
            nc.vector.tensor_scalar_mul(
                woT[:, s, rt * 128:(rt + 1) * 128], pt, rmsc[:, (s % 2):(s % 2) + 1]
            )

    # ---- state ----
    z256 = const.tile([128, HV], F32)
    nc.vector.memset(z256, 0.0)
    S = []
    for h in range(HPC):
        st = sS.tile([128, HV], F32R, tag="S")
        nc.scalar.copy(st, z256)
        S.append(st)

    AF = mybir.AluOpType
    ACT = mybir.ActivationFunctionType

    lbstate = {"prev_hT": None}

    def stage_lb(lb):
        # hT block: [:, d, 8:8+LB] = fresh transposed h; [:, d, 5:8] = prev tail
        hT = hpool.tile([128, KD, LB + 8], F32R, tag="hT")
        if lb > 0:
            nc.vector.tensor_copy(hT[:, :, 5:8], lbstate["prev_hT"][:, :, LB + 5:LB + 8])
        for lt in range(LB // 128):
            hr = hrow.tile([128, D], F32, tag="hrow")
            row = lb * (LB // 128) + lt
            nc.sync.dma_start(out=hr, in_=hs[row * 128:(row + 1) * 128, :])
            for d in range(KD):
                pt = transpose_f32(hr[:, d * 128:(d + 1) * 128])
                copy_ps(hT[:, d, 8 + lt * 128:8 + (lt + 1) * 128], pt)
        lbstate["prev_hT"] = hT

        # conv + silu -> xT block [128, KD, LB]
        xT = xpool.tile([128, KD, LB], F32R, tag="xT")
        for d in range(KD):
            dg = dpool.tile([128, CONV, 128], F32R, tag="dg")
            for j in range(CONV):
                nc.scalar.mul(dg[:, j, :], ident, cwt[:, d * CONV + j:d * CONV + j + 1])
            pc = ps.tile([128, LB], F32, tag="ps")
            if lb == 0:
                # first block: clip the shifted taps instead of zero-padding
                _mm(nc, pc, dg[:, 3, :], hT[:, d, 8:8 + LB], start=True, stop=False)
                for j in range(3):
                    nc.tensor.matmul(
                        pc[:, 3 - j:LB], dg[:, j, :].bitcast(F32),
                        hT[:, d, 8:8 + LB - (3 - j)].bitcast(F32),
                        start=False, stop=(j == 2))
            else:
                for j in range(CONV):
                    _mm(nc, pc, dg[:, j, :], hT[:, d, 5 + j:5 + j + LB],
                        start=(j == 0), stop=(j == CONV - 1))
            nc.scalar.activation(xT[:, d, :], pc, ACT.Silu)

        # q/k projections (T layout) for this L-block
        qT = qk.tile([128, HPC, LB], F32R, tag="qT")
        kT = qk.tile([128, HPC, LB], F32, tag="kT")
        for h in range(HPC):
            for (wt, dst) in ((wqT, qT), (wkT, kT)):
                pp = ps.tile([128, LB], F32, tag="ps")
                for ks in range(KD):
                    _mm(nc, pp, wt[:, ks, h * 128:(h + 1) * 128], xT[:, ks, :],
                        start=(ks == 0), stop=(ks == KD - 1))
                copy_ps(dst[:, h, :], pp)
        return qT, kT, xT

    def stage_a(c, qT, kT, xT):
        """Chunk-parallel work: v/g/beta projections, k-norm, A/Mqk, TinvT, -W^T."""
        ch = c % (LB // C)
        csl = slice(ch * C, (ch + 1) * C)

        pv0 = ps.tile([128, HV + HPC], F32, tag="ps")
        pv1 = ps.tile([128, HV], F32, tag="ps")
        pg = ps.tile([128, HPC * HV], F32, tag="ps")
        for ks in range(KD):
            lx = xT[:, ks, csl]
            _mm(nc, pv0, lx, wvbT[:, ks, :], start=(ks == 0), stop=(ks == KD - 1))
            _mm(nc, pv1, lx, wvT1[:, ks, :], start=(ks == 0), stop=(ks == KD - 1))
            _mm(nc, pg, lx, wgT[:, ks, :], start=(ks == 0), stop=(ks == KD - 1))
        beta = sm.tile([128, HPC], F32, tag="beta")
        nc.scalar.activation(beta, pv0[:, HV:HV + HPC], ACT.Sigmoid)
        sg = cv.tile([128, HPC * HV], F32, tag="sg")
        nc.scalar.activation(sg, pg, ACT.Silu)
        vb = cv.tile([128, HPC * HV], F32R, tag="vb")
        nc.vector.tensor_scalar_mul(vb[:, 0:HV], pv0[:, 0:HV], beta[:, 0:1])
        nc.vector.tensor_scalar_mul(vb[:, HV:2 * HV], pv1, beta[:, 1:2])

        art = {"vb": vb, "sg": sg, "qT": qT, "csl": csl, "h": []}
        for h in range(HPC):
            # --- k normalization (row space) ---
            pt = transpose_f32(kT[:, h, csl])
            kraw = ck.tile([128, 128], F32, tag="kraw")
            copy_ps(kraw, pt)
            sq = scr.tile([128, 128], F32, tag="sq")
            nsq = sm.tile([128, 1], F32, tag="nsq")
            nc.scalar.activation(sq, kraw, ACT.Square, accum_out=nsq)
            nrm = sm.tile([128, 1], F32, tag="nrm")
            nc.scalar.sqrt(nrm, nsq)
            nrm2 = sm.tile([128, 1], F32, tag="nrm2")
            nc.vector.tensor_scalar_max(nrm2, nrm, 1e-6)
            inv = sm.tile([128, 1], F32, tag="inv")
            nc.vector.reciprocal(inv, nrm2)
            knr = ckx.tile([128, 128], F32R, tag="knr")   # Kn row [C, HK]
            nc.vector.tensor_scalar_mul(knr, kraw, inv)
            kbr = ck.tile([128, 128], F32, tag="kbr")    # beta*Kn row
            nc.vector.tensor_scalar_mul(kbr, knr.bitcast(F32), beta[:, h:h + 1])
            pt = transpose_f32(knr.bitcast(F32))
            knT = ck.tile([128, 128], F32, tag="knT")
            copy_ps(knT, pt)
            pt = transpose_f32(kbr)
            kbT = ck.tile([128, 128], F32, tag="kbT")
            copy_ps(kbT, pt)

            # --- A^T = Kn Kb^T ; Mqk^T = masked Kn Q^T ---
            pA = ps.tile([128, 128], F32, tag="ps")
            nc.tensor.matmul(pA, knT, kbT, start=True, stop=True)
            pM = ps.tile([128, 128], F32, tag="ps")
            nc.tensor.matmul(pM, knT, qT[:, h, csl].bitcast(F32), start=True, stop=True)
            mqk = ckx.tile([128, 128], F32R, tag="mqk")
            nc.vector.tensor_mul(mqk, pM, umask)

            # --- TinvT = sum_k M^k, M = strict_upper(-A^T), bf16 doubling ---
            Mb = ck.tile([128, 128], BF16, tag="Mb")
            nc.vector.tensor_mul(Mb, pA, numask)
            S2 = ck.tile([128, 128], BF16, tag="S2")
            nc.vector.tensor_add(S2, Mb, identb)
            pt = ps.tile([128, 128], BF16, tag="ps")
            nc.tensor.transpose(pt, Mb, identb)
            Nb = ck.tile([128, 128], BF16, tag="Nb")
            copy_ps(Nb, pt)

            def mmb(lhsT, rhs):
                po = ps.tile([128, 128], F32, tag="ps")
                nc.tensor.matmul(po, lhsT, rhs, start=True, stop=True)
                return po

            def cast_b(po, tag):
                t = ck.tile([128, 128], BF16, tag=tag)
                copy_ps(t, po)
                return t

            P2 = cast_b(mmb(Nb, Mb), "P2")     # M @ M
            P2T = cast_b(mmb(Mb, Nb), "P2T")   # (M @ M)^T
            S4 = ck.tile([128, 128], BF16, tag="S4")
            nc.vector.tensor_add(S4, S2, mmb(P2T, S2))
            P4T = cast_b(mmb(P2, P2T), "P4T")
            if NEUMANN16:
                S8 = ck.tile([128, 128], BF16, tag="S8")
                nc.vector.tensor_add(S8, S4, mmb(P4T, S4))
                P4 = cast_b(mmb(P2T, P2), "P4")
                P8T = cast_b(mmb(P4, P4T), "P8T")
                tinvT = ckx.tile([128, 128], F32R, tag="tinvT")
                nc.vector.tensor_add(tinvT, S8, mmb(P8T, S8))
            else:
                tinvT = ckx.tile([128, 128], F32R, tag="tinvT")
                nc.vector.tensor_add(tinvT, S4, mmb(P4T, S4))

            # --- -W^T = -(Kb^T Tinv^T) ---
            pW = ps.tile([128, 128], F32, tag="ps")
            nc.tensor.matmul(pW, kbr, tinvT.bitcast(F32), start=True, stop=True)
            nWT = ckx.tile([128, 128], F32R, tag="nWT")
            nc.scalar.mul(nWT, pW, -1.0)
            art["h"].append({"knr": knr, "mqk": mqk, "tinvT": tinvT, "nWT": nWT})
        return art

    def stage_b(c, art):
        """S-dependent sequential phase + gated rmsnorm + output projection."""
        vb, sg, qT, csl = art["vb"], art["sg"], art["qT"], art["csl"]
        ofin = cv.tile([128, HPC * HV], F32, tag="ofin")
        for h in range(HPC):
            hsl = slice(h * HV, (h + 1) * HV)
            a = art["h"][h]
            # --- U = Tinv Vb - W S ---
            pU = ps.tile([128, HV], F32, tag="ps")
            _mm(nc, pU, a["nWT"], S[h], start=True, stop=False)
            _mm(nc, pU, a["tinvT"], vb[:, hsl], start=False, stop=True)
            U = cv.tile([128, HV], F32R, tag="U")
            copy_ps(U, pU)

            # --- O = Q S + Mqk U ---
            pO = ps.tile([128, HV], F32, tag="ps")
            _mm(nc, pO, qT[:, h, csl], S[h], start=True, stop=False)
            _mm(nc, pO, a["mqk"], U, start=False, stop=True)

            # --- gated rmsnorm: ofin = (O * rsqrt(mean O^2 + eps)) * silu(g)
            sq2 = scr.tile([128, HV], F32, tag="sq2")
            ms = sm.tile([128, 1], F32, tag="ms")
            nc.scalar.activation(sq2, pO, ACT.Square, accum_out=ms)
            rs1 = sm.tile([128, 1], F32, tag="rs1")
            nc.scalar.activation(rs1, ms, ACT.Sqrt, bias=epst, scale=1.0 / HV)
            rs = sm.tile([128, 1], F32, tag="rs")
            nc.vector.reciprocal(rs, rs1)
            nc.vector.scalar_tensor_tensor(
                out=ofin[:, hsl], in0=pO, scalar=rs, in1=sg[:, hsl],
                op0=AF.mult, op1=AF.mult,
            )

            # --- S += Kn^T U ---
            pD = ps.tile([128, HV], F32, tag="ps")
            _mm(nc, pD, a["knr"], U, start=True, stop=True)
            Sn = sS.tile([128, HV], F32R, tag="S")
            nc.vector.tensor_add(Sn, S[h].bitcast(F32), pD)
            S[h] = Sn

        # --- partial output projection: y[c] = ofin @ woT ---
        oT = otp.tile([128, 4, 128], F32R, tag="oT")
        for s in range(4):
            pt = transpose_f32(ofin[:, s * 128:(s + 1) * 128])
            copy_ps(oT[:, s, :], pt)
        for t2 in range(2):
            py = ps.tile([128, 512], F32, tag="ps")
            for s in range(4):
                _mm(nc, py, oT[:, s, :], woT[:, s, t2 * 512:(t2 + 1) * 512],
                    start=(s == 0), stop=(s == 3))
            yst = cv.tile([128, 512], F32, tag="yst")
            copy_ps(yst, py)
            nc.sync.dma_start(
                out=y[c * 128:(c + 1) * 128, t2 * 512:(t2 + 1) * 512], in_=yst
            )

    # software pipeline: stage A of chunk c+1 is emitted before stage B of
    # chunk c, so the PE always has independent work while the sequential
    # S-chain of the previous chunk waits on DVE/ACT results.
    CPB = LB // C
    arts = {}
    cur = None
    for c in range(NCH + 1):
        if c < NCH:
            if c % CPB == 0:
                cur = stage_lb(c // CPB)
            arts[c] = stage_a(c, *cur)
        if c >= 1:
            stage_b(c - 1, arts.pop(c - 1))

    ctx.close()


_nc_cache = None


def _get_nc():
    global _nc_cache
    if _nc_cache is None:
        _nc_cache = build_program()
    return _nc_cache


def make_in_maps(hidden_states, conv_w, Wq, Wk, Wv, Wb, Wg, Wo, rms_weight):
    arr = lambda a: np.ascontiguousarray(np.asarray(a, dtype=np.float32))
    in_maps = []
    for core in range(N_CORES):
        b, g = core // 2, core % 2
        in_maps.append({
            "hs": arr(hidden_states[b]),
            "cw": arr(conv_w),
            "wq": arr(Wq[g * HPC * HK:(g + 1) * HPC * HK]),
            "wk": arr(Wk[g * HPC * HK:(g + 1) * HPC * HK]),
            "wv": arr(Wv[g * HPC * HV:(g + 1) * HPC * HV]),
            "wb": arr(Wb[g * HPC:(g + 1) * HPC]),
            "wg": arr(Wg[g * HPC * HV:(g + 1) * HPC * HV]),
            "wo": arr(Wo[:, g * HPC * HV:(g + 1) * HPC * HV]),
            "rmsw": arr(rms_weight),
        })
    return in_maps


def unshard(results):
    y = np.empty((B, L, D), np.float32)
    for b in range(B):
        y[b] = results[2 * b]["y"] + results[2 * b + 1]["y"]
    return y


def kernel(hidden_states, conv_w, Wq, Wk, Wv, Wb, Wg, Wo, rms_weight, **_ignored):
    nc = _get_nc()
    in_maps = make_in_maps(hidden_states, conv_w, Wq, Wk, Wv, Wb, Wg, Wo, rms_weight)
    res = run_bass_kernel_spmd(nc, in_maps, core_ids=list(range(N_CORES)))
    return unshard(res.results)



# revision 10
# speedup vs baseline: 1.4429x; 1.4429x over previous
"""DeltaNet forward kernel for Trainium2, sharded over 8 NeuronCores.

Sharding: core c handles batch c//2 and head-pair c%2 (heads {2*(c%2), 2*(c%2)+1}).
Host pre-transposes hidden_states to [D, L+3] (3 zero pad tokens for the causal
conv) and ships all weights pre-transposed/scaled in their SBUF layouts, so the
device does no weight staging and no input transposes. Each core computes:
causal depthwise conv+silu (via diagonal-matrix matmuls), q/k/v/beta/g
projections, the delta-rule recurrence via the chunked WY representation
(chunk=128, (I+A)^-1 via Neumann-series doubling in bf16), gated RMSNorm, and a
partial output projection against its 512-column slice of Wo (rms_weight folded
in). The host sums the two half-DV partials per batch (row-parallel unshard).
"""

import sys

for _p in ("/opt/trn_rl_repo", "/root/.axon_site"):
    if _p not in sys.path:
        sys.path.insert(0, _p)

import numpy as np

import concourse.bass as bass
import concourse.tile as tile
from concourse import bacc, mybir
from concourse.bass_utils import run_bass_kernel_spmd
from concourse.masks import make_identity

F32 = mybir.dt.float32
F32R = mybir.dt.float32r
BF16 = mybir.dt.bfloat16

B, L, D, H = 4, 2048, 1024, 4
DK, DV = 512, 1024
HK, HV = 128, 256
CONV, EPS = 4, 1e-5
C = 128            # delta-rule chunk length
NCH = L // C       # 16 chunks
LB = 256           # L-block for conv/q projections
NLB = L // LB      # 8
KD = D // 128      # 8 contraction slices
HPC = 2            # heads per core
N_CORES = 8
QSCALE = HK ** -0.5


def build_program():
    nc = bacc.Bacc(
        "TRN2", target_bir_lowering=False, debug=False,
        enable_asserts=False, num_devices=N_CORES,
    )

    hst = nc.dram_tensor("hst", [D, L + 3], F32, kind="ExternalInput").ap()
    dgt = nc.dram_tensor("dgt", [128, KD * CONV, 128], F32, kind="ExternalInput").ap()
    wqt = nc.dram_tensor("wqt", [128, KD, HPC * HK], F32, kind="ExternalInput").ap()
    wkbt = nc.dram_tensor("wkbt", [128, KD, HPC * HK + HPC], F32, kind="ExternalInput").ap()
    wvt = nc.dram_tensor("wvt", [128, KD, HPC * HV], F32, kind="ExternalInput").ap()
    wgt = nc.dram_tensor("wgt", [128, KD, HPC * HV], F32, kind="ExternalInput").ap()
    wot = nc.dram_tensor("wot", [128, 4, D], F32, kind="ExternalInput").ap()
    y = nc.dram_tensor("y", [L, D], F32, kind="ExternalOutput").ap()

    with tile.TileContext(nc) as tc:
        _build_body(nc, tc, hst, dgt, wqt, wkbt, wvt, wgt, wot, y)
    nc.compile()
    return nc


def _build_body(nc, tc, hst, dgt, wqt, wkbt, wvt, wgt, wot, y):
    from contextlib import ExitStack

    ctx = ExitStack()
    const = ctx.enter_context(tc.tile_pool(name="const", bufs=1))
    wT = ctx.enter_context(tc.tile_pool(name="wT", bufs=1))
    ps = ctx.enter_context(tc.tile_pool(name="ps", bufs=1, space="PSUM"))
    hpool = ctx.enter_context(tc.tile_pool(name="hpool", bufs=2))
    xpool = ctx.enter_context(tc.tile_pool(name="xpool", bufs=2))
    qk = ctx.enter_context(tc.tile_pool(name="qk", bufs=2))
    scr = ctx.enter_context(tc.tile_pool(name="scr", bufs=3))
    ck = ctx.enter_context(tc.tile_pool(name="ck", bufs=3))
    ckx = ctx.enter_context(tc.tile_pool(name="ckx", bufs=3))
    cv = ctx.enter_context(tc.tile_pool(name="cv", bufs=2))
    otp = ctx.enter_context(tc.tile_pool(name="otp", bufs=2))
    sS = ctx.enter_context(tc.tile_pool(name="sS", bufs=2))
    sm = ctx.enter_context(tc.tile_pool(name="sm", bufs=4))

    AF = mybir.AluOpType
    ACT = mybir.ActivationFunctionType

    def r(ap):
        return ap.bitcast(F32R) if ap.dtype == F32 else ap

    def mm(out, lhsT, rhs, start, stop):
        nc.tensor.matmul(out, r(lhsT), r(rhs), start=start, stop=stop)

    # ---- constants ----
    identf = const.tile([128, 128], F32)
    make_identity(nc, identf)
    identr = const.tile([128, 128], F32R)
    nc.vector.tensor_copy(identr, identf)
    identb = const.tile([128, 128], BF16)
    make_identity(nc, identb)
    epst = const.tile([128, 1], F32)
    nc.vector.memset(epst, EPS)
    # umask: 1 where free >= part (upper incl diag); numask: -1 where free > part
    umask = const.tile([128, 128], F32)
    nc.gpsimd.memset(umask, 1.0)
    nc.gpsimd.affine_select(
        out=umask, in_=umask, compare_op=mybir.AluOpType.is_ge, fill=0.0,
        base=0, channel_multiplier=-1, pattern=[[1, 128]],
    )
    numask = const.tile([128, 128], F32)
    nc.gpsimd.memset(numask, -1.0)
    nc.gpsimd.affine_select(
        out=numask, in_=numask, compare_op=mybir.AluOpType.is_gt, fill=0.0,
        base=0, channel_multiplier=-1, pattern=[[1, 128]],
    )

    def transpose_r(in_):
        pt = ps.tile([128, 128], F32R, tag="p0", bufs=4)
        nc.tensor.transpose(pt, r(in_), identr)
        return pt.bitcast(F32)

    # ---- weight loads (pre-transposed on host) ----
    dg = wT.tile([128, KD * CONV, 128], F32R)
    nc.sync.dma_start(out=dg, in_=dgt.bitcast(F32R))
    wqT = wT.tile([128, KD, HPC * HK], F32R)
    nc.sync.dma_start(out=wqT, in_=wqt.bitcast(F32R))
    wkbT = wT.tile([128, KD, HPC * HK + HPC], F32R)
    nc.sync.dma_start(out=wkbT, in_=wkbt.bitcast(F32R))
    wvT = wT.tile([128, KD, HPC * HV], F32R)
    nc.sync.dma_start(out=wvT, in_=wvt.bitcast(F32R))
    wgT = wT.tile([128, KD, HPC * HV], F32R)
    nc.sync.dma_start(out=wgT, in_=wgt.bitcast(F32R))
    woT = wT.tile([128, 4, D], F32R)
    nc.sync.dma_start(out=woT, in_=wot.bitcast(F32R))

    # ---- state: S for both heads in one [128, 512] tile ----
    z512 = const.tile([128, HPC * HV], F32)
    nc.vector.memset(z512, 0.0)
    S0 = sS.tile([128, HPC * HV], F32R, tag="S")
    nc.vector.tensor_copy(S0, z512)
    Sref = [S0]

    def stage_lb(lb):
        # hT: [128, KD, LB+3]; token axis holds t0-3 .. t0+LB-1 (host zero-pads)
        hT = hpool.tile([128, KD, LB + 3], F32R, tag="hT")
        for d in range(KD):
            nc.sync.dma_start(
                out=hT[:, d, :], in_=hst[d * 128:(d + 1) * 128, lb * LB:lb * LB + LB + 3].bitcast(F32R)
            )

        # conv + silu -> xT [128, KD, LB]; two d-slices share one psum bank
        xT = xpool.tile([128, KD, LB], F32R, tag="xT")
        for dp in range(KD // 2):
            pc = ps.tile([128, 2 * LB], F32, tag="p2", bufs=4)
            for dd in range(2):
                d = 2 * dp + dd
                for j in range(CONV):
                    mm(pc[:, dd * LB:(dd + 1) * LB], dg[:, d * CONV + j, :],
                       hT[:, d, j:j + LB], start=(j == 0), stop=(j == CONV - 1))
            nc.scalar.activation(xT[:, 2 * dp:2 * dp + 2, :],
                                 pc.rearrange("p (d t) -> p d t", d=2), ACT.Silu)

        # qT [128, HPC, LB]: both heads' accumulation groups in one psum tile
        qT = qk.tile([128, HPC, LB], F32R, tag="qT")
        for half in range(LB // C):
            pp = ps.tile([128, HPC * C], F32, tag="p2", bufs=4)
            for h in range(HPC):
                for ks in range(KD):
                    mm(pp[:, h * C:(h + 1) * C],
                       wqT[:, ks, h * 128:(h + 1) * 128],
                       xT[:, ks, half * C:(half + 1) * C],
                       start=(ks == 0), stop=(ks == KD - 1))
            nc.vector.tensor_copy(qT[:, :, half * C:(half + 1) * C],
                                  pp.rearrange("p (h c) -> p h c", h=HPC))
        return qT, xT

    def stage_a(c, qT, xT):
        """Chunk-parallel work: k/v/g/beta projections, k-norm, A/Mqk, TinvT, -W^T."""
        ch = c % (LB // C)
        csl = slice(ch * C, (ch + 1) * C)

        pk = ps.tile([128, HPC * HK + HPC], F32, tag="p2", bufs=4)
        pv = ps.tile([128, HPC * HV], F32, tag="p2", bufs=4)
        pg = ps.tile([128, HPC * HV], F32, tag="p2", bufs=4)
        for ks in range(KD):
            lx = xT[:, ks, csl]
            mm(pk, lx, wkbT[:, ks, :], start=(ks == 0), stop=(ks == KD - 1))
            mm(pv, lx, wvT[:, ks, :], start=(ks == 0), stop=(ks == KD - 1))
            mm(pg, lx, wgT[:, ks, :], start=(ks == 0), stop=(ks == KD - 1))
        beta = sm.tile([128, HPC], F32, tag="beta")
        nc.scalar.activation(beta, pk[:, HPC * HK:HPC * HK + HPC], ACT.Sigmoid)
        sg = cv.tile([128, HPC * HV], F32, tag="sg")
        nc.scalar.activation(sg, pg, ACT.Silu)
        vb = cv.tile([128, HPC * HV], F32R, tag="vb")
        nc.vector.tensor_scalar_mul(vb[:, 0:HV], pv[:, 0:HV], beta[:, 0:1])
        nc.vector.tensor_scalar_mul(vb[:, HV:2 * HV], pv[:, HV:2 * HV], beta[:, 1:2])

        art = {"vb": vb, "sg": sg, "qT": qT, "csl": csl, "h": []}
        for h in range(HPC):
            ksl = slice(h * HK, (h + 1) * HK)
            # --- k normalization (row space, straight from psum) ---
            sq = scr.tile([128, 128], F32, tag="sq")
            nsq = sm.tile([128, 1], F32, tag="nsq")
            nc.scalar.activation(sq, pk[:, ksl], ACT.Square, accum_out=nsq)
            nrm = sm.tile([128, 1], F32, tag="nrm")
            nc.scalar.sqrt(nrm, nsq)
            nrm2 = sm.tile([128, 1], F32, tag="nrm2")
            nc.vector.tensor_scalar_max(nrm2, nrm, 1e-6)
            inv = sm.tile([128, 1], F32, tag="inv")
            nc.vector.reciprocal(inv, nrm2)
            knr = ckx.tile([128, 128], F32R, tag="knr")   # Kn row [C, HK]
            nc.vector.tensor_scalar_mul(knr, pk[:, ksl], inv)
            kbr = ck.tile([128, 128], F32R, tag="kbr")    # beta*Kn row
            nc.vector.tensor_scalar_mul(kbr, knr.bitcast(F32), beta[:, h:h + 1])
            knT = ck.tile([128, 128], F32R, tag="knT")
            nc.vector.tensor_copy(knT, transpose_r(knr))
            kbT = ck.tile([128, 128], F32R, tag="kbT")
            nc.vector.tensor_copy(kbT, transpose_r(kbr))

            # --- A^T = Kn Kb^T ; Mqk^T = masked Kn Q^T ---
            pA = ps.tile([128, 128], F32, tag="p0", bufs=4)
            mm(pA, knT, kbT, start=True, stop=True)
            pM = ps.tile([128, 128], F32, tag="p0", bufs=4)
            mm(pM, knT, qT[:, h, csl], start=True, stop=True)
            mqk = ckx.tile([128, 128], F32R, tag="mqk")
            nc.vector.tensor_mul(mqk, pM, umask)

            # --- TinvT = sum_{k<16} M^k, M = strict_upper(-A^T), bf16 doubling ---
            Mb = ck.tile([128, 128], BF16, tag="Mb")
            nc.vector.tensor_mul(Mb, pA, numask)
            S2 = ck.tile([128, 128], BF16, tag="S2")
            nc.vector.tensor_add(S2, Mb, identb)
            ptb = ps.tile([128, 128], BF16, tag="p0", bufs=4)
            nc.tensor.transpose(ptb, Mb, identb)
            Nb = ck.tile([128, 128], BF16, tag="Nb")
            nc.vector.tensor_copy(Nb, ptb)

            def mmb(lhsT, rhs):
                po = ps.tile([128, 128], F32, tag="p0", bufs=4)
                nc.tensor.matmul(po, lhsT, rhs, start=True, stop=True)
                return po

            def cast_b(po, tag):
                t = ck.tile([128, 128], BF16, tag=tag)
                nc.vector.tensor_copy(t, po)
                return t

            P2 = cast_b(mmb(Nb, Mb), "P2")     # M @ M
            P2T = cast_b(mmb(Mb, Nb), "P2T")   # (M @ M)^T
            S4 = ck.tile([128, 128], BF16, tag="S4")
            nc.vector.tensor_add(S4, S2, mmb(P2T, S2))
            P4T = cast_b(mmb(P2, P2T), "P4T")
            S8 = ck.tile([128, 128], BF16, tag="S8")
            nc.vector.tensor_add(S8, S4, mmb(P4T, S4))
            P4 = cast_b(mmb(P2T, P2), "P4")
            P8T = cast_b(mmb(P4, P4T), "P8T")
            tinvT = ckx.tile([128, 128], F32R, tag="tinvT")
            nc.vector.tensor_add(tinvT, S8, mmb(P8T, S8))

            # --- -W^T = -(Kb^T Tinv^T) ---
            pW = ps.tile([128, 128], F32, tag="p0", bufs=4)
            mm(pW, kbr, tinvT, start=True, stop=True)
            nWT = ckx.tile([128, 128], F32R, tag="nWT")
            nc.vector.tensor_scalar_mul(nWT, pW, -1.0)
            art["h"].append({"knr": knr, "mqk": mqk, "tinvT": tinvT, "nWT": nWT})
        return art

    def stage_b(c, art):
        """S-dependent sequential phase + gated rmsnorm + output projection."""
        vb, sg, qT, csl = art["vb"], art["sg"], art["qT"], art["csl"]
        S = Sref[0]
        ofin = cv.tile([128, HPC * HV], F32R, tag="ofin")
        pU = ps.tile([128, HPC * HV], F32, tag="p2", bufs=4)
        for h in range(HPC):
            hsl = slice(h * HV, (h + 1) * HV)
            a = art["h"][h]
            # --- U = Tinv Vb - W S ---
            mm(pU[:, hsl], a["nWT"], S[:, hsl], start=True, stop=False)
            mm(pU[:, hsl], a["tinvT"], vb[:, hsl], start=False, stop=True)
        U = cv.tile([128, HPC * HV], F32R, tag="U")
        nc.vector.tensor_copy(U, pU)

        pO = ps.tile([128, HPC * HV], F32, tag="p2", bufs=4)
        pD = ps.tile([128, HPC * HV], F32, tag="p2", bufs=4)
        for h in range(HPC):
            hsl = slice(h * HV, (h + 1) * HV)
            a = art["h"][h]
            # --- O = Q S + Mqk U ---
            mm(pO[:, hsl], qT[:, h, csl], S[:, hsl], start=True, stop=False)
            mm(pO[:, hsl], a["mqk"], U[:, hsl], start=False, stop=True)
            # --- S += Kn^T U ---
            mm(pD[:, hsl], a["knr"], U[:, hsl], start=True, stop=True)

            # --- gated rmsnorm: ofin = (O * rsqrt(mean O^2 + eps)) * silu(g)
            sq2 = scr.tile([128, HV], F32, tag="sq2")
            ms = sm.tile([128, 1], F32, tag="ms")
            nc.scalar.activation(sq2, pO[:, hsl], ACT.Square, accum_out=ms)
            rs1 = sm.tile([128, 1], F32, tag="rs1")
            nc.scalar.activation(rs1, ms, ACT.Sqrt, bias=epst, scale=1.0 / HV)
            rs = sm.tile([128, 1], F32, tag="rs")
            nc.vector.reciprocal(rs, rs1)
            nc.vector.scalar_tensor_tensor(
                out=ofin[:, hsl], in0=pO[:, hsl], scalar=rs, in1=sg[:, hsl],
                op0=AF.mult, op1=AF.mult,
            )

        Sn = sS.tile([128, HPC * HV], F32R, tag="S")
        nc.vector.tensor_add(Sn, S.bitcast(F32), pD)
        Sref[0] = Sn

        # --- partial output projection: y[c] = ofin @ woT ---
        oT = otp.tile([128, 4, 128], F32R, tag="oT")
        for s in range(4):
            nc.vector.tensor_copy(oT[:, s, :], transpose_r(ofin[:, s * 128:(s + 1) * 128]))
        yst = cv.tile([128, D], F32, tag="yst")
        for t2 in range(2):
            py = ps.tile([128, 512], F32, tag="p2", bufs=4)
            for s in range(4):
                mm(py, oT[:, s, :], woT[:, s, t2 * 512:(t2 + 1) * 512],
                   start=(s == 0), stop=(s == 3))
            nc.vector.tensor_copy(yst[:, t2 * 512:(t2 + 1) * 512], py)
        nc.sync.dma_start(out=y[c * 128:(c + 1) * 128, :], in_=yst)

    # software pipeline: stage A of chunk c+1 is emitted before stage B of
    # chunk c, so the PE always has independent work while the sequential
    # S-chain of the previous chunk waits on DVE/ACT results.
    CPB = LB // C
    arts = {}
    cur = None
    for c in range(NCH + 1):
        if c < NCH:
            if c % CPB == 0:
                cur = stage_lb(c // CPB)
            arts[c] = stage_a(c, *cur)
        if c >= 1:
            stage_b(c - 1, arts.pop(c - 1))

    ctx.close()


_nc_cache = None


def _get_nc():
    global _nc_cache
    if _nc_cache is None:
        _nc_cache = build_program()
    return _nc_cache


def _wt(w):
    """[F, D] weight -> [128, KD, F] transposed tile layout, flattened."""
    f = w.shape[0]
    t = np.ascontiguousarray(w.T.reshape(KD, 128, f).transpose(1, 0, 2))
    return t.reshape(128, KD * f)


def make_in_maps(hidden_states, conv_w, Wq, Wk, Wv, Wb, Wg, Wo, rms_weight):
    arr = lambda a: np.ascontiguousarray(np.asarray(a, dtype=np.float32))
    hidden_states = np.asarray(hidden_states, np.float32)
    conv_w = np.asarray(conv_w, np.float32)
    Wq, Wk, Wv, Wb = (np.asarray(w, np.float32) for w in (Wq, Wk, Wv, Wb))
    Wg, Wo = np.asarray(Wg, np.float32), np.asarray(Wo, np.float32)
    rms_weight = np.asarray(rms_weight, np.float32)

    # conv diag tiles: dgt[p, (d*CONV+j), f] = (p==f) * cw[d*128+p, j]
    dgt = np.zeros((128, KD * CONV, 128), np.float32)
    idx = np.arange(128)
    for d in range(KD):
        for j in range(CONV):
            dgt[idx, d * CONV + j, idx] = conv_w[d * 128:(d + 1) * 128, j]
    dgt = dgt.reshape(128, KD * CONV * 128)

    hst = {}
    for b in range(B):
        hp = np.pad(hidden_states[b], ((3, 0), (0, 0)))      # [L+3, D]
        hst[b] = np.ascontiguousarray(hp.T)                  # [D, L+3]

    in_maps = []
    for core in range(N_CORES):
        b, g = core // 2, core % 2
        wq_g = Wq[g * HPC * HK:(g + 1) * HPC * HK] * QSCALE  # [256, D]
        wk_g = Wk[g * HPC * HK:(g + 1) * HPC * HK]           # [256, D]
        wb_g = Wb[g * HPC:(g + 1) * HPC]                     # [2, D]
        wkb_g = np.concatenate([wk_g, wb_g], axis=0)         # [258, D]
        wv_g = Wv[g * HPC * HV:(g + 1) * HPC * HV]           # [512, D]
        wg_g = Wg[g * HPC * HV:(g + 1) * HPC * HV]           # [512, D]
        # wot[p, s, f] = Wo[f, g*512 + s*128 + p] * rmsw[(s%2)*128 + p]
        wo_g = Wo[:, g * HPC * HV:(g + 1) * HPC * HV]        # [D, 512]
        wo_t = wo_g.T.reshape(4, 128, D)                     # [s, p, f]
        rmsf = rms_weight.reshape(2, 128)[np.array([0, 1, 0, 1])]  # [4, 128]
        wo_t = np.ascontiguousarray(
            (wo_t * rmsf[:, :, None]).transpose(1, 0, 2)).reshape(128, 4 * D)
        in_maps.append({
            "hst": hst[b],
            "dgt": dgt,
            "wqt": arr(_wt(wq_g)),
            "wkbt": arr(_wt(wkb_g)),
            "wvt": arr(_wt(wv_g)),
            "wgt": arr(_wt(wg_g)),
            "wot": arr(wo_t),
        })
    return in_maps


def unshard(results):
    y = np.empty((B, L, D), np.float32)
    for b in range(B):
        y[b] = results[2 * b]["y"] + results[2 * b + 1]["y"]
    return y


def kernel(hidden_states, conv_w, Wq, Wk, Wv, Wb, Wg, Wo, rms_weight, **_ignored):
    nc = _get_nc()
    in_maps = make_in_maps(hidden_states, conv_w, Wq, Wk, Wv, Wb, Wg, Wo, rms_weight)
    res = run_bass_kernel_spmd(nc, in_maps, core_ids=list(range(N_CORES)))
    return unshard(res.results)
